# revision 1
# baseline (speedup 1.0000x reference)
"""Complex coherency loss, distributed over 8 TRN2 NeuronCores.

Data-parallel over batch: core b computes the partial coherency sum for
batch element b; the host sums the per-chunk partials and finishes the
mean.

v2 design (vs the DMA-cast baseline):
  - The host converts inputs to bf16 and packs all four tensors into ONE
    [128, 4N] array per core (parity layout p = 2c + l%2, n = l//2), so
    the kernel reads 8.4 MB instead of 16.8 MB -- the load window halves
    to the ~22 us HBM roofline.  One plain HWDGE DMA per group.
  - 5 moving tensors into the PE channel-reduction instead of 8:
    m12s = pr*tr + pi*ti (pre-summed on DVE), m3/m4 kept separate for
    the sign, pa = pr^2+pi^2, ta = tr^2+ti^2 pre-summed.  PE work drops
    from 27.5 us to ~17 us.
  - PSUM [8, fd] is drained by ACT directly into SBUF staging tiles
    (bf16) -- no DRAM staging round-trip.
  - The [8, N] -> [P', windows] re-partition runs as SBUF->SBUF DMAs on
    the gpsimd (SWDGE) queue, which nothing else uses.
  - The k=5 sliding-window sum runs on the PE as 5 identity-weight
    accumulate-matmuls per parity into a PSUM win tile; only the ratio
    (7 small ops) stays on DVE.
  - 4 tail chunks fire as their staging columns land, so the final chunk
    (64 partitions x 4 windows) is tiny.
  - Output DMAs ride the scalar (ACT) HWDGE ring; final out is [128, 4]
    per-chunk accumulator columns, summed on the host.
"""

import numpy as np
import ml_dtypes

import concourse.bass as bass
import concourse.bacc as bacc
import concourse.mybir as mybir
import concourse.tile as tile
from concourse.bass_utils import run_bass_kernel_spmd

B, C, L = 8, 64, 16384
K = 5
P = 128
N = (C * L) // P          # 8192 position pairs per core
NVALID = L - K + 1        # 16380
CH = 512                  # matmul moving-dim chunk (one PSUM bank of f32)

GROUP_FDS = [512, 1024, 1536, 1536, 1536, 1024, 772, 252]
assert sum(GROUP_FDS) == N
GROUP_ENDS = list(np.cumsum(GROUP_FDS))
FD_MAX = max(GROUP_FDS)

# Tail chunks: (n0, npp, Pn, W).  Partition p' of chunk c holds window
# pairs n = n0 + npp*p' + i for i in [0, W); the halo tile carries W+4
# columns.  Chunk 0 fires mid-stream; chunk 3 is tiny so the post-load
# tail is short.  Windows n >= N-2 are invalid (masked, chunk 3 only).
CHUNKS = [
    (0,    64, 95, 64),
    (6080, 16, 64, 16),
    (7104, 13, 64, 13),
    (7936,  4, 64,  4),
]
STG_SPANS = [(n0, n0 + npp * (Pn - 1) + W + 4) for n0, npp, Pn, W in CHUNKS]
assert STG_SPANS[-1][1] == N + 4
STG_W = N + 4             # staging row width (4 zero-pad columns)

F32 = mybir.dt.float32
BF16 = mybir.dt.bfloat16

PROFILE = False
TRACE_DIR = None
LAST_RESULT = None


def _selector_weights() -> np.ndarray:
    """Five [128, 8] weight matrices, packed as [128, 40] bf16.

    Matrix w maps a moving tensor into PSUM rows 2q+par (par = p % 2):
      w=0: m12s -> rows 0,1 (+)   w=1: m3 -> rows 2,3 (+)
      w=2: m4   -> rows 2,3 (-)   w=3: pa -> rows 4,5 (+)
      w=4: ta   -> rows 6,7 (+)
    """
    w = np.zeros((P, 5 * 8), dtype=np.float32)
    p = np.arange(P)
    h = p % 2
    w[p, 0 * 8 + 0 + h] = 1.0
    w[p, 1 * 8 + 2 + h] = 1.0
    w[p, 2 * 8 + 2 + h] = -1.0
    w[p, 3 * 8 + 4 + h] = 1.0
    w[p, 4 * 8 + 6 + h] = 1.0
    return w.astype(ml_dtypes.bfloat16)


def build_nc() -> bacc.Bacc:
    nc = bacc.Bacc("TRN2", target_bir_lowering=False, debug=False)

    in_d = nc.dram_tensor("inp", [P, 4 * N], BF16, kind="ExternalInput").ap()
    out_d = nc.dram_tensor("out", [P, 4], F32, kind="ExternalOutput").ap()
    w_d = nc.inline_tensor(_selector_weights(), name="selw").ap()
    eye_d = nc.inline_tensor(
        np.eye(P, dtype=ml_dtypes.bfloat16), name="eye"
    ).ap()
    # Chunk-3 validity mask over flat [par, w] = [2, 4]: window pairs
    # n = 7936 + 4*63 + i are invalid for i in {2, 3}.
    mask_np = np.ones((64, 8), dtype=np.float32)
    mask_np[63, 2:4] = 0.0
    mask_np[63, 6:8] = 0.0
    mask_d = nc.inline_tensor(mask_np, name="mask").ap()

    with tile.TileContext(nc) as tc:
        with (
            tc.tile_pool(name="consts", bufs=1) as consts,
            tc.tile_pool(name="ins", bufs=1) as ins,
            tc.tile_pool(name="prods", bufs=2) as prods,
            tc.tile_pool(name="drt", bufs=2) as drt,
            tc.tile_pool(name="fin", bufs=1) as fin,
            tc.tile_pool(name="psum", bufs=2, space="PSUM") as psum,
            tc.tile_pool(name="dram", bufs=1, space="DRAM") as dram,
        ):
            w_sb = consts.tile([P, 5 * 8], BF16)
            nc.sync.dma_start(w_sb[:, :], w_d)
            eye_sb = consts.tile([P, P], BF16)
            nc.sync.dma_start(eye_sb[:, :], eye_d)

            # Pre-warm the Sqrt activation table off the critical path.
            warm = consts.tile([P, 1], F32)
            nc.vector.memset(warm[:, :], 1.0)
            nc.scalar.sqrt(warm[:, :], warm[:, :])

            mask8 = consts.tile([64, 8], F32)
            nc.sync.dma_start(mask8[:, :], mask_d)

            # DRAM staging, bf16: row r = 2q + par, column n holds the
            # channel sum of quantity q at position l = 2n + par.
            stg = dram.tile([8, STG_W], BF16)
            zeros = consts.tile([1, 8 * (STG_W - N)], BF16)
            nc.vector.memset(zeros[:, :], 0.0)
            nc.sync.dma_start(stg[:, N:STG_W], zeros[:, :])

            # Preload all input groups (plain bf16 HWDGE DMAs, FIFO on
            # the SP ring so groups complete in order).  Host layout is
            # group-major: per group, a contiguous (pr|pi) block then a
            # contiguous (tr|ti) block, so each DMA is fully contiguous
            # per partition (large descriptors, line-rate).
            tins = []
            col = 0
            for g, fd in enumerate(GROUP_FDS):
                t_p = ins.tile([P, 2 * fd], BF16, name=f"tp{g}")
                t_t = ins.tile([P, 2 * fd], BF16, name=f"tt{g}")
                for j, t in enumerate((t_p, t_t)):
                    src = bass.AP(
                        tensor=in_d.tensor,
                        offset=4 * col + j * 2 * fd,
                        ap=[[4 * N, P], [1, 2 * fd]],
                    )
                    nc.sync.dma_start(t[:, :], src)
                tins.append((t_p, t_t))
                col += fd

            # Squares ride DVE (2X mode, 1.92 Gcol/s) for some groups and
            # ACT (1.0 Gcol/s) for the rest to balance the two engines.
            # All 8 product tensors stream raw into the PE (no pre-adds;
            # PE runs 2.4 Gcol/s once its p-state ramp is warm).
            sq_eng = {0: "act", 1: "act", 2: "act", 3: "act",
                      4: "dve", 5: "dve", 6: "dve", 7: "dve"}
            group_state = {}

            def emit_products(g):
                fd = GROUP_FDS[g]
                t_p, t_t = tins[g]
                pr, pi = t_p[:, 0:fd], t_p[:, fd:2 * fd]
                tr, ti = t_t[:, 0:fd], t_t[:, fd:2 * fd]

                def ptile(nm):
                    return prods.tile([P, fd], BF16, name=nm, tag=nm,
                                      padded_shape=[P, FD_MAX])
                m1, m2 = ptile("m1"), ptile("m2")
                m3, m4 = ptile("m3"), ptile("m4")
                nc.vector.tensor_mul(m1[:, :], pr, tr)
                nc.vector.tensor_mul(m2[:, :], pi, ti)
                nc.vector.tensor_mul(m3[:, :], pi, tr)
                nc.vector.tensor_mul(m4[:, :], pr, ti)

                sqa, sqb = ptile("sqa"), ptile("sqb")
                sqc, sqd = ptile("sqc"), ptile("sqd")
                if sq_eng[g] == "act":
                    nc.scalar.square(sqa[:, :], pr)
                    nc.scalar.square(sqb[:, :], pi)
                    nc.scalar.square(sqc[:, :], tr)
                    nc.scalar.square(sqd[:, :], ti)
                else:
                    nc.vector.tensor_mul(sqa[:, :], pr, pr)
                    nc.vector.tensor_mul(sqb[:, :], pi, pi)
                    nc.vector.tensor_mul(sqc[:, :], tr, tr)
                    nc.vector.tensor_mul(sqd[:, :], ti, ti)

                group_state[g] = [
                    (0, m1), (0, m2), (1, m3), (2, m4),
                    (3, sqa), (3, sqb), (4, sqc), (4, sqd),
                ]

            def emit_mm_drain(g):
                fd = GROUP_FDS[g]
                c0 = GROUP_ENDS[g] - fd
                streams = group_state.pop(g)
                ps = psum.tile([8, fd], F32, name="ps", tag="ps",
                               padded_shape=[8, FD_MAX])
                nstr = len(streams)
                for si, (widx, mov) in enumerate(streams):
                    lhsT = w_sb[:, widx * 8:(widx + 1) * 8]
                    for kk in range(0, fd, CH):
                        ks = slice(kk, min(kk + CH, fd))
                        nc.tensor.matmul(
                            ps[:, ks], lhsT, mov[:, ks],
                            start=(si == 0), stop=(si == nstr - 1),
                        )
                # drain PSUM -> bf16 SBUF relay -> DRAM staging
                dr = drt.tile([8, fd], BF16, name="dr", tag="dr",
                              padded_shape=[8, FD_MAX])
                nc.scalar.activation(
                    dr[:, :], ps[:, :], mybir.ActivationFunctionType.Copy
                )
                nc.sync.dma_start(stg[:, c0:c0 + fd], dr[:, :])

            chunk_halos = {}

            def emit_chunk_halos(ci):
                n0, npp, Pn, W = CHUNKS[ci]
                H = W + 4
                halos = []
                for par in range(2):
                    h = fin.tile([Pn, 4 * H], BF16, name=f"halo{ci}{par}")
                    src = bass.AP(
                        tensor=stg.tensor,
                        offset=stg.offset + par * STG_W + n0,
                        ap=[[npp, Pn], [2 * STG_W, 4], [1, H]],
                    )
                    nc.gpsimd.dma_start(
                        h.rearrange("p (q i) -> p q i", q=4), src
                    )
                    halos.append(h.rearrange("p (q i) -> p q i", q=4))
                chunk_halos[ci] = halos

            def emit_chunk_win(ci):
                """winE = E0+E1+E2+O0+O1 ; winO = O0+O1+O2+E1+E2.
                Chunk 0 (large) runs 10 identity accumulate-matmuls on
                the idle PE + an ACT copy to SBUF; small chunks run bf16
                shifted adds on DVE straight into an SBUF win tile."""
                n0, npp, Pn, W = CHUNKS[ci]
                hE, hO = chunk_halos[ci]
                if ci == 0:
                    win = psum.tile([Pn, 2 * 4 * W], F32, name=f"win{ci}",
                                    tag="win", padded_shape=[P, 512])
                    eye = eye_sb[0:Pn, 0:Pn]
                    for par, (h0, h1) in enumerate(((hE, hO), (hO, hE))):
                        shifts = [(h0, 0), (h0, 1), (h0, 2)] + (
                            [(h1, 0), (h1, 1)] if par == 0
                            else [(h1, 1), (h1, 2)]
                        )
                        reg = win[:, par * 4 * W:(par + 1) * 4 * W]
                        for si, (hh, j) in enumerate(shifts):
                            nc.tensor.matmul(
                                reg, eye, hh[:, :, j:j + W],
                                start=(si == 0), stop=(si == 4),
                            )
                    winS = fin.tile([Pn, 2 * 4 * W], F32, name=f"winS{ci}")
                    nc.scalar.activation(
                        winS[:, :], win[:, :],
                        mybir.ActivationFunctionType.Copy,
                    )
                else:
                    winS = fin.tile([Pn, 2 * 4 * W], BF16, name=f"winS{ci}")
                    wv = winS.rearrange("p (r q w) -> p r q w", r=2, q=4)
                    for par, (h0, h1) in enumerate(((hE, hO), (hO, hE))):
                        js = [(h1, 0), (h1, 1)] if par == 0 \
                            else [(h1, 1), (h1, 2)]
                        w4 = wv[:, par]
                        nc.vector.tensor_add(
                            w4, h0[:, :, 0:W], h0[:, :, 1:W + 1])
                        nc.vector.tensor_add(w4, w4, h0[:, :, 2:W + 2])
                        for hh, j in js:
                            nc.vector.tensor_add(w4, w4, hh[:, :, j:j + W])
                return winS

            def emit_chunk_ratio(ci, winS):
                n0, npp, Pn, W = CHUNKS[ci]

                def winq(q):
                    return bass.AP(
                        tensor=winS.tensor,
                        offset=winS.offset + q * W,
                        ap=[list(winS.ap[0]), [4 * W, 2], [1, W]],
                    )
                wr, wi, wa, wt = winq(0), winq(1), winq(2), winq(3)
                n2 = fin.tile([Pn, 2 * W], F32, name=f"n2_{ci}")
                t2 = fin.tile([Pn, 2 * W], F32, name=f"t2_{ci}")
                d2 = fin.tile([Pn, 2 * W], F32, name=f"d2_{ci}")
                rd = fin.tile([Pn, 2 * W], F32, name=f"rd_{ci}")
                n2v = n2.rearrange("p (r w) -> p r w", r=2)
                t2v = t2.rearrange("p (r w) -> p r w", r=2)
                d2v = d2.rearrange("p (r w) -> p r w", r=2)
                nc.vector.tensor_mul(n2v, wr, wr)
                nc.vector.tensor_mul(t2v, wi, wi)
                nc.vector.tensor_add(n2[:, :], n2[:, :], t2[:, :])
                nc.vector.tensor_mul(d2v, wa, wt)
                nc.vector.reciprocal(rd[:, :], d2[:, :])
                nc.vector.tensor_mul(n2[:, :], n2[:, :], rd[:, :])
                if ci == 3:
                    nc.vector.tensor_mul(n2[:, :], n2[:, :], mask8[:, :])
                sq = fin.tile([Pn, 2 * W], F32, name=f"sq{ci}")
                acc = fin.tile([Pn, 1], F32, name=f"acc{ci}")
                nc.scalar.activation(
                    sq[:, :], n2[:, :],
                    mybir.ActivationFunctionType.Sqrt,
                    accum_out=acc[:, :],
                )
                nc.scalar.dma_start(out_d[0:Pn, ci:ci + 1], acc[:, :])

            # Emission: halo DMAs go out right after the last group whose
            # staging columns the chunk needs (they ride the otherwise
            # idle gpsimd queue); all chunk engine work is emitted after
            # the full main loop so it never blocks the drains or the PE
            # stream in their FIFOs.
            halo_after = {4: 0, 5: 1, 6: 2, 7: 3}
            for g in range(len(GROUP_FDS)):
                emit_products(g)
                emit_mm_drain(g)
                if g in halo_after:
                    emit_chunk_halos(halo_after[g])
            w0 = emit_chunk_win(0)
            emit_chunk_ratio(0, w0)
            for ci in (1, 2, 3):
                wS = emit_chunk_win(ci)
                emit_chunk_ratio(ci, wS)

    nc.compile()
    return nc


_NC = None


def _get_nc() -> bacc.Bacc:
    global _NC
    if _NC is None:
        _NC = build_nc()
    return _NC


def kernel(pred_real, pred_imag, targ_real, targ_imag, filter_size=5):
    global LAST_RESULT
    assert int(filter_size) == K
    nc = _get_nc()

    bf = ml_dtypes.bfloat16
    in_maps = []
    for b in range(B):
        pvs = []
        for x in (pred_real[b], pred_imag[b], targ_real[b], targ_imag[b]):
            x = np.asarray(x, dtype=np.float32)
            # parity layout: partition 2c + (l%2), free n = l//2
            pvs.append(x.reshape(C, N, 2).transpose(0, 2, 1).reshape(P, N))
        # group-major packing: per group (pr|pi) block then (tr|ti) block
        arr = np.empty((P, 4 * N), dtype=bf)
        off = c0 = 0
        for fd in GROUP_FDS:
            for j in range(4):
                arr[:, off + j * fd:off + (j + 1) * fd] = \
                    pvs[j][:, c0:c0 + fd]
            off += 4 * fd
            c0 += fd
        in_maps.append({"inp": arr})

    kwargs = {}
    if PROFILE:
        kwargs = dict(trace=True)
        if TRACE_DIR is not None:
            import os
            os.makedirs(TRACE_DIR, exist_ok=True)
            kwargs["tmpdir"] = TRACE_DIR
    res = run_bass_kernel_spmd(nc, in_maps, core_ids=list(range(B)), **kwargs)
    LAST_RESULT = res

    total = 0.0
    for r in res.results:
        o = np.asarray(r["out"], dtype=np.float64)
        for ci, (n0, npp, Pn, W) in enumerate(CHUNKS):
            total += o[0:Pn, ci].sum()
    coh = total / (B * NVALID)
    return np.float32(1.0 - coh)



# revision 26
# speedup vs baseline: 1.0029x; 1.0029x over previous
"""Complex coherency loss, distributed over 8 TRN2 NeuronCores.

Data-parallel over batch: core b handles batch element b; the host sums
the per-chunk partial sums and finishes the mean.

v3 design (vs the v2 parity/bf16 kernel):
  - fp8 (e4m3) end to end: inputs ship as 4.2 MB per core (half of v2's
    bf16), and the channel reduction runs as DoubleRow fp8 matmuls (two
    128-deep k-tiles per pass) at 2x the bf16 column rate.
  - comp-interleaved layout: partition p = 2c + (re/im).  Group g covers
    l in [2048g, 2048g+2048); its two 1024-col k-tiles are routed to
    separate PSUM rows by the selector weights, so PSUM comes out in
    natural l order (no parity split anywhere).
  - products: M = P*T (one DVE mul covers pr*tr and pi*ti), M2 = P*T_sw
    (T re-read from HBM with comp-swapped rows) with a +/- selector for
    the imaginary part, plus ACT/DVE squares of P and T for |p|^2, |t|^2.
  - PSUM [8, 1024] f32 drains straight to a flat DRAM scratch [4, L+4]
    via plain HWDGE DMAs (no engine time), and stage 2 gathers
    [128, 4, 68] window tiles with one DMA per chunk.
  - stage 2 is flat: 4 shifted adds per quantity, ratio, sqrt-accum.
  - PE warmup matmuls during the load window ride out the p-state ramp.
"""

import numpy as np
import ml_dtypes

import concourse.bass as bass
import concourse.bacc as bacc
import concourse.mybir as mybir
import concourse.tile as tile
from concourse.bass_utils import run_bass_kernel_spmd

B, C, L = 8, 64, 16384
K = 5
P = 128
NVALID = L - K + 1        # 16380 valid windows per batch element

NG = 8                    # groups
FD = 1024                 # cols per k-tile per group
GSPAN = 2 * FD            # l-span per group
ROWL = 2 * L              # inp row length (fp8 cols per partition)
SCR_W = L + 4             # scratch cols per quantity (4 zero-pad cols)
NQ = 4                    # m12, m34, A, B
CH = 512                  # matmul chunk (psum bank, f32)
NPP = 64                  # l's per partition in stage 2
HALO = 4

F32 = mybir.dt.float32
BF16 = mybir.dt.bfloat16
FP8 = mybir.dt.float8e4
NP_FP8 = ml_dtypes.float8_e4m3

PROFILE = False
TRACE_DIR = None
LAST_RESULT = None


MROWS = 128               # matmul output rows (dual-fp8 ldweights must
                          # load the full PE array width)


def _selector_weights() -> np.ndarray:
    """Per-stream selectors, packed [128, NQ * 2 * MROWS] fp8.  Stream s
    routes its k-tile i into PSUM row m = 4*i + s and nothing else.
    m34's sign: partition 2c holds pr*ti (negative), 2c+1 pi*tr
    (positive)."""
    w = np.zeros((P, NQ, 2, MROWS), dtype=np.float32)
    p = np.arange(P)
    sign = np.where(p % 2 == 1, 1.0, -1.0)
    for s in range(NQ):
        for i in range(2):
            w[p, s, i, 4 * i + s] = sign if s == 1 else 1.0
    return w.reshape(P, NQ * 2 * MROWS).astype(NP_FP8)


def _fuse_ldweights(nc) -> None:
    """Re-fuse Ldweights+Matmult into self-loading matmuls.

    Tile legalization splits every matmul into a standalone Ldweights
    followed by a Matmult(ldweights=False), but this walrus build rejects
    standalone DoubleRow Ldweights ("not compatible with LDW
    optimization").  Self-loading DoubleRow matmuls codegen fine, so
    merge each pair back: union the waits, inherit dependencies, drop
    the Ldweights."""
    rename = {}
    for fn in nc.m.functions:
        for blk in fn.blocks:
            ins = list(blk.instructions)
            out, pending = [], None
            for inst in ins:
                if inst.opcode == "Ldweights":
                    assert pending is None
                    pending = inst
                    continue
                if inst.opcode == "Matmult" and pending is not None:
                    lsi, si = pending.sync_info, inst.sync_info
                    lw_waits = list(lsi.on_wait) if lsi else []
                    mm_waits = list(si.on_wait) if si else []
                    mm_upd = list(si.on_update) if si else []
                    inst.sync_info = mybir.SyncInfo(
                        on_wait=lw_waits + mm_waits, on_update=mm_upd)
                    inst.ldweights = True
                    inst.merge_dependencies_from(pending)
                    rename[pending.name] = inst.name
                    pending = None
                out.append(inst)
            assert pending is None
            blk.instructions = out
    if rename:
        for fn in nc.m.functions:
            for blk in fn.blocks:
                for inst in blk.instructions:
                    inst.remap_dependency_names(rename)


def build_nc() -> bacc.Bacc:
    nc = bacc.Bacc("TRN2", target_bir_lowering=False, debug=False)

    in_d = nc.dram_tensor("inp", [P, ROWL], FP8, kind="ExternalInput").ap()
    out_d = nc.dram_tensor("out", [P, 2], F32, kind="ExternalOutput").ap()
    w_d = nc.inline_tensor(_selector_weights(), name="selw").ap()
    mask_np = np.ones((P, NPP), dtype=ml_dtypes.bfloat16)
    mask_np[P - 1, NPP - 4:NPP] = 0.0
    mask_d = nc.inline_tensor(mask_np, name="mask").ap()
    DR = mybir.MatmulPerfMode.DoubleRow

    with tile.TileContext(nc) as tc:
        with (
            tc.tile_pool(name="consts", bufs=1) as consts,
            tc.tile_pool(name="ins", bufs=1) as ins,
            tc.tile_pool(name="prods", bufs=2) as prods,
            tc.tile_pool(name="fin", bufs=1) as fin,
            tc.tile_pool(name="psum", bufs=2, space="PSUM") as psum,
            tc.tile_pool(name="wps", bufs=1, space="PSUM") as wps,
            tc.tile_pool(name="dram", bufs=1, space="DRAM") as dram,
        ):
            # ---- input loads first: 2 HWDGE DMAs per group on the SP ring.
            tins = []
            for g in range(NG):
                t_in = ins.tile([P, 4 * FD], FP8, name=f"in{g}")
                src = bass.AP(
                    tensor=in_d.tensor,
                    offset=g * 4 * FD,
                    ap=[[ROWL, P], [1, 4 * FD]],
                )
                nc.sync.dma_start(t_in[:, :], src)
                # T re-read with comp-swapped rows: out row 2c+j reads
                # row 2c+(1-j) of the T block.
                t_sw = ins.tile([P, 2 * FD], FP8, name=f"sw{g}")
                src_sw = bass.AP(
                    tensor=in_d.tensor,
                    offset=ROWL + g * 4 * FD + 2 * FD,
                    ap=[[2 * ROWL, 64], [-ROWL, 2], [1, 2 * FD]],
                )
                nc.sync.dma_start(t_sw[:, :], src_sw)
                tins.append((t_in, t_sw))

            # ---- consts ride the ACT hwdge queue; memsets on gpsimd.
            w_sb = consts.tile([P, NQ * 2 * MROWS], FP8)
            nc.scalar.dma_start(w_sb[:, :], w_d)
            w4 = w_sb.rearrange("p (s i m) -> p s i m", s=NQ, i=2)
            wsel = [w4[:, s] for s in range(NQ)]

            scr = dram.tile([NQ, SCR_W], BF16)
            zeros = consts.tile([1, 16], BF16)
            nc.gpsimd.memset(zeros[:, :], 0.0)
            scr_pad = bass.AP(
                tensor=scr.tensor,
                offset=scr.offset + L,
                ap=[[SCR_W, NQ], [1, HALO]],
            )
            nc.sync.dma_start(scr_pad, zeros[:, :])

            # stage-2 mask: zero the 4 invalid windows (l >= L-4).
            mask = consts.tile([P, NPP], BF16)
            nc.scalar.dma_start(mask[:, :], mask_d)

            # ---- PE p-state warmup during the load window.
            wtile = consts.tile([P, 2 * CH], FP8)
            nc.gpsimd.memset(wtile[:, :], 0.25)
            wview = wtile.rearrange("p (i n) -> p i n", i=2)
            ps_w = wps.tile([MROWS, CH], F32)
            for _ in range(20):
                nc.tensor.matmul(
                    ps_w[:, :], wsel[0], wview, start=True, stop=True,
                    perf_mode=DR,
                )

            # pre-warm ACT's sqrt table off the critical path.
            warm = consts.tile([P, 1], F32)
            nc.gpsimd.memset(warm[:, :], 1.0)
            nc.scalar.sqrt(warm[:, :], warm[:, :])

            # ---- main loop.
            def emit_group(g):
                t_in, t_sw = tins[g]
                p_t = t_in[:, 0:2 * FD]
                tt = t_in[:, 2 * FD:4 * FD]

                def ptile(nm):
                    return prods.tile([P, 2 * FD], FP8, name=nm, tag=nm)
                m, m2 = ptile("m"), ptile("m2")
                aq, bq = ptile("aq"), ptile("bq")
                nc.vector.tensor_mul(m[:, :], p_t, tt)
                nc.scalar.square(aq[:, :], p_t)
                nc.scalar.square(bq[:, :], tt)
                nc.gpsimd.tensor_mul(m2[:, :], p_t, t_sw[:, :])

                ps = psum.tile([MROWS, FD], F32, name="ps", tag="ps")
                movs = (m, m2, aq, bq)
                for si, mov in enumerate(movs):
                    mv = mov.rearrange("p (i n) -> p i n", i=2)
                    for kk in range(0, FD, CH):
                        nc.tensor.matmul(
                            ps[:, kk:kk + CH], wsel[si], mv[:, :, kk:kk + CH],
                            start=(si == 0), stop=(si == len(movs) - 1),
                            perf_mode=DR,
                        )
                # drain psum -> bf16 SBUF relay (split ACT/DVE), then DMA
                # rows 4i..4i+3 -> scr[q, 2048g + 1024i + n]
                dr = prods.tile([8, FD], BF16, name="dr", tag="dr")
                nc.vector.tensor_copy(dr[:, :], ps[0:8, :])
                for i in range(2):
                    dst = bass.AP(
                        tensor=scr.tensor,
                        offset=scr.offset + g * GSPAN + i * FD,
                        ap=[[SCR_W, NQ], [1, FD]],
                    )
                    nc.sync.dma_start(dst, dr[4 * i:4 * i + 4, :])

            def emit_chunk(c):
                stg = fin.tile([P, NQ * (NPP + HALO)], BF16, name=f"stg{c}")
                sv = stg.rearrange("p (q n) -> p q n", q=NQ)
                src = bass.AP(
                    tensor=scr.tensor,
                    offset=scr.offset + c * P * NPP,
                    ap=[[NPP, P], [SCR_W, NQ], [1, NPP + HALO]],
                )
                nc.sync.dma_start(sv, src)

                win = fin.tile([P, NQ * NPP], BF16, name=f"win{c}")
                wv = win.rearrange("p (q n) -> p q n", q=NQ)
                nc.vector.tensor_add(wv, sv[:, :, 0:NPP], sv[:, :, 1:NPP + 1])
                for j in range(2, K):
                    nc.vector.tensor_add(wv, wv, sv[:, :, j:NPP + j])

                def winq(q):
                    return win[:, q * NPP:(q + 1) * NPP]
                n2 = fin.tile([P, NPP], BF16, name=f"n2_{c}")
                t2 = fin.tile([P, NPP], BF16, name=f"t2_{c}")
                d2 = fin.tile([P, NPP], BF16, name=f"d2_{c}")
                rd = fin.tile([P, NPP], F32, name=f"rd_{c}")
                nc.vector.tensor_mul(n2[:, :], winq(0), winq(0))
                nc.vector.tensor_mul(t2[:, :], winq(1), winq(1))
                nc.vector.tensor_add(n2[:, :], n2[:, :], t2[:, :])
                nc.vector.tensor_mul(d2[:, :], winq(2), winq(3))
                nc.vector.reciprocal(rd[:, :], d2[:, :])
                nc.vector.tensor_mul(n2[:, :], n2[:, :], rd[:, :])
                if c == 1:
                    nc.vector.tensor_mul(n2[:, :], n2[:, :], mask[:, :])
                sq = fin.tile([P, NPP], F32, name=f"sq{c}")
                acc = fin.tile([P, 1], F32, name=f"acc{c}")
                nc.scalar.activation(
                    sq[:, :], n2[:, :],
                    mybir.ActivationFunctionType.Sqrt,
                    accum_out=acc[:, :],
                )
                nc.scalar.dma_start(out_d[:, c:c + 1], acc[:, :])

            for g in range(NG):
                emit_group(g)
                if g == 4:
                    emit_chunk(0)
            emit_chunk(1)

    _fuse_ldweights(nc)
    nc.compile()
    return nc


_NC = None


def _get_nc() -> bacc.Bacc:
    global _NC
    if _NC is None:
        _NC = build_nc()
    return _NC


def kernel(pred_real, pred_imag, targ_real, targ_imag, filter_size=5):
    global LAST_RESULT
    assert int(filter_size) == K
    nc = _get_nc()

    in_maps = []
    for b in range(B):
        # comp-interleave: partition 2c + (0 for real, 1 for imag)
        pm = np.empty((P, L), dtype=np.float32)
        tm = np.empty((P, L), dtype=np.float32)
        pm[0::2] = np.asarray(pred_real[b], dtype=np.float32)
        pm[1::2] = np.asarray(pred_imag[b], dtype=np.float32)
        tm[0::2] = np.asarray(targ_real[b], dtype=np.float32)
        tm[1::2] = np.asarray(targ_imag[b], dtype=np.float32)
        pm8 = pm.astype(NP_FP8)
        tm8 = tm.astype(NP_FP8)
        # group-blocked: per group, P block (2048 cols) then T block.
        arr = np.empty((P, ROWL), dtype=NP_FP8)
        a3 = arr.reshape(P, NG, 4 * FD)
        a3[:, :, 0:GSPAN] = pm8.reshape(P, NG, GSPAN)
        a3[:, :, GSPAN:4 * FD] = tm8.reshape(P, NG, GSPAN)
        in_maps.append({"inp": arr})

    kwargs = {}
    if PROFILE:
        kwargs = dict(trace=True)
        if TRACE_DIR is not None:
            import os
            os.makedirs(TRACE_DIR, exist_ok=True)
            kwargs["tmpdir"] = TRACE_DIR
    res = run_bass_kernel_spmd(nc, in_maps, core_ids=list(range(B)), **kwargs)
    LAST_RESULT = res

    total = 0.0
    for r in res.results:
        total += np.asarray(r["out"], dtype=np.float64).sum()
    coh = total / (B * NVALID)
    return np.float32(1.0 - coh)


# revision 29
# speedup vs baseline: 1.0275x; 1.0245x over previous
"""Complex coherency loss, distributed over 8 TRN2 NeuronCores.

Data-parallel over batch: core b handles batch element b; the host sums
the per-chunk partial sums and finishes the mean.

v4 design (bf16 plane-separate; replaces the fp8 DoubleRow v3):
  - measured v3 showed fp8 elementwise is slow on HW (no DVE 2x mode for
    1-byte dtypes, GpSimd mul ~6us/2048cols) and starved fp8 matmuls
    never reach the DoubleRow rate.  bf16 keeps DVE in 2x mode.
  - plane-separate layout: partition p = c + 64h (h = l-half), plane
    tiles PR/PI/TR/TI [128, 1024] per group.  All four products
    (pr*tr, pi*ti, pi*tr, pr*ti) are direct tile muls - no swapped copy
    of T and no extra HBM traffic.
  - 8 moving streams per group into the PE; selector weights route
    stream q to PSUM row 2q+h (m4 with weight -1), accumulating m12,
    m34, A, B in natural l order within each half.
  - PSUM [8, 1024] f32 -> DVE/ACT/GPS-split bf16 relay -> 2 scatter
    DMAs per group into a flat DRAM scratch [4, L+4].
  - stage 2 in 4 chunks of 4096 windows ([64, 4, 68] gathers): flat
    5-tap window adds, ratio, sqrt-accum.  Two chunks fire mid-loop.
  - PE p-state warmup matmuls during the load window.
"""

import numpy as np
import ml_dtypes

import concourse.bass as bass
import concourse.bacc as bacc
import concourse.mybir as mybir
import concourse.tile as tile
from concourse.bass_utils import run_bass_kernel_spmd

B, C, L = 8, 64, 16384
K = 5
P = 128
NVALID = L - K + 1        # 16380 valid windows per batch element

NG = 8                    # groups
FD = 1024                 # plane cols per group (per half)
HL = L // 2               # 8192 l's per half
SCR_W = L + 4             # scratch cols per quantity (4 zero-pad cols)
NQ = 4                    # m12, m34, A, B
NS = 8                    # moving streams per group
CH = 512                  # matmul chunk (one psum bank of f32)
NPP = 64                  # l's per partition in stage 2
CK = 64 * NPP             # 4096 l's per stage-2 chunk
HALO = 4

F32 = mybir.dt.float32
BF16 = mybir.dt.bfloat16
NP_BF16 = ml_dtypes.bfloat16

PROFILE = False
TRACE_DIR = None
LAST_RESULT = None

# stream -> (quantity row q, sign): m1,m2->m12; m3,+ m4,- ->m34; squares
STREAM_Q = [(0, 1.0), (0, 1.0), (1, 1.0), (1, -1.0),
            (2, 1.0), (2, 1.0), (3, 1.0), (3, 1.0)]


def _selector_weights() -> np.ndarray:
    """Per-stream selectors [128, NS * 8] bf16: stream s routes
    partition p = c + 64h into PSUM row 4h + q_s with weight sign_s
    (h-major rows so each half drains as a contiguous row block)."""
    w = np.zeros((P, NS, 8), dtype=np.float32)
    p = np.arange(P)
    h = p // 64
    for s, (q, sign) in enumerate(STREAM_Q):
        w[p, s, 4 * h + q] = sign
    return w.reshape(P, NS * 8).astype(NP_BF16)


def _fuse_ldweights(nc) -> None:
    """Re-fuse Ldweights+Matmult into self-loading matmuls (tile
    legalization splits them; merging back halves PE instruction count
    and sidesteps walrus restrictions on standalone Ldweights)."""
    rename = {}
    for fn in nc.m.functions:
        for blk in fn.blocks:
            ins = list(blk.instructions)
            out, pending = [], None
            for inst in ins:
                if inst.opcode == "Ldweights":
                    assert pending is None
                    pending = inst
                    continue
                if inst.opcode == "Matmult" and pending is not None:
                    lsi, si = pending.sync_info, inst.sync_info
                    lw_waits = list(lsi.on_wait) if lsi else []
                    mm_waits = list(si.on_wait) if si else []
                    mm_upd = list(si.on_update) if si else []
                    inst.sync_info = mybir.SyncInfo(
                        on_wait=lw_waits + mm_waits, on_update=mm_upd)
                    inst.ldweights = True
                    inst.merge_dependencies_from(pending)
                    rename[pending.name] = inst.name
                    pending = None
                out.append(inst)
            assert pending is None
            blk.instructions = out
    if rename:
        for fn in nc.m.functions:
            for blk in fn.blocks:
                for inst in blk.instructions:
                    inst.remap_dependency_names(rename)


def build_nc() -> bacc.Bacc:
    nc = bacc.Bacc("TRN2", target_bir_lowering=False, debug=False)

    in_d = nc.dram_tensor("inp", [P, 4 * HL], BF16, kind="ExternalInput").ap()
    out_d = nc.dram_tensor("out", [P, 4], F32, kind="ExternalOutput").ap()
    w_d = nc.inline_tensor(_selector_weights(), name="selw").ap()
    mask_np = np.ones((64, NPP), dtype=NP_BF16)
    mask_np[63, NPP - 4:NPP] = 0.0
    mask_d = nc.inline_tensor(mask_np, name="mask").ap()

    with tile.TileContext(nc) as tc:
        with (
            tc.tile_pool(name="consts", bufs=1) as consts,
            tc.tile_pool(name="ins", bufs=1) as ins,
            tc.tile_pool(name="prods", bufs=2) as prods,
            tc.tile_pool(name="fin", bufs=1) as fin,
            tc.tile_pool(name="psum", bufs=2, space="PSUM") as psum,
            tc.tile_pool(name="wps", bufs=1, space="PSUM") as wps,
            tc.tile_pool(name="dram", bufs=1, space="DRAM") as dram,
        ):
            # ---- input loads first: 1 HWDGE DMA per group on the SP ring.
            tins = []
            for g in range(NG):
                t_in = ins.tile([P, 4 * FD], BF16, name=f"in{g}")
                src = bass.AP(
                    tensor=in_d.tensor,
                    offset=g * 4 * FD,
                    ap=[[4 * HL, P], [1, 4 * FD]],
                )
                nc.sync.dma_start(t_in[:, :], src)
                tins.append(t_in)

            # ---- consts ride the ACT hwdge queue; memsets on gpsimd.
            w_sb = consts.tile([P, NS * 8], BF16)
            nc.scalar.dma_start(w_sb[:, :], w_d)
            w3 = w_sb.rearrange("p (s m) -> p s m", s=NS)
            wsel = [w3[:, s] for s in range(NS)]

            scr = dram.tile([NQ, SCR_W], BF16)
            zeros = consts.tile([1, 16], BF16)
            nc.gpsimd.memset(zeros[:, :], 0.0)
            scr_pad = bass.AP(
                tensor=scr.tensor,
                offset=scr.offset + L,
                ap=[[SCR_W, NQ], [1, HALO]],
            )
            nc.sync.dma_start(scr_pad, zeros[:, :])

            # stage-2 mask: zero the 4 invalid windows (l >= L-4).
            mask = consts.tile([64, NPP], BF16)
            nc.scalar.dma_start(mask[:, :], mask_d)

            # ---- PE p-state warmup during the load window.
            wtile = consts.tile([P, CH], BF16)
            nc.gpsimd.memset(wtile[:, :], 0.25)
            ps_w = wps.tile([8, CH], F32)
            for _ in range(24):
                nc.tensor.matmul(
                    ps_w[:, :], wsel[0], wtile[:, :], start=True, stop=True,
                )

            # pre-warm ACT's sqrt table off the critical path.
            warm = consts.tile([P, 1], F32)
            nc.gpsimd.memset(warm[:, :], 1.0)
            nc.scalar.sqrt(warm[:, :], warm[:, :])

            # ---- main loop.
            def emit_group(g):
                t_in = tins[g]
                pl = [t_in[:, j * FD:(j + 1) * FD] for j in range(4)]
                pr, pi, tr, ti = pl

                def ptile(nm):
                    return prods.tile([P, FD], BF16, name=nm, tag=nm)
                movs = [ptile(f"s{s}") for s in range(NS)]
                # DVE: the four cross products (bf16 2x mode)
                nc.vector.tensor_mul(movs[0][:, :], pr, tr)
                nc.vector.tensor_mul(movs[1][:, :], pi, ti)
                nc.vector.tensor_mul(movs[2][:, :], pi, tr)
                nc.vector.tensor_mul(movs[3][:, :], pr, ti)
                # ACT: three squares; GPS: one square
                nc.scalar.square(movs[4][:, :], pr)
                nc.scalar.square(movs[5][:, :], pi)
                nc.scalar.square(movs[6][:, :], tr)
                nc.gpsimd.tensor_mul(movs[7][:, :], ti, ti)

                ps = psum.tile([8, FD], F32, name="ps", tag="ps")
                for si in range(NS):
                    for kk in range(0, FD, CH):
                        nc.tensor.matmul(
                            ps[:, kk:kk + CH], wsel[si],
                            movs[si][:, kk:kk + CH],
                            start=(si == 0), stop=(si == NS - 1),
                        )
                # drain psum -> bf16 relay (split DVE/ACT), then 2 scatter
                # DMAs: row 4h+q -> scr[q, 8192h + 1024g + n]
                dr = prods.tile([8, FD], BF16, name="dr", tag="dr")
                nc.vector.tensor_copy(dr[:, 0:CH], ps[:, 0:CH])
                nc.scalar.copy(dr[:, CH:FD], ps[:, CH:FD])
                for h in range(2):
                    dst = bass.AP(
                        tensor=scr.tensor,
                        offset=scr.offset + h * HL + g * FD,
                        ap=[[SCR_W, NQ], [1, FD]],
                    )
                    nc.sync.dma_start(dst, dr[4 * h:4 * h + 4, :])

            def emit_chunk(c):
                l0 = (c % 2) * HL + (c // 2) * CK
                stg = fin.tile([64, NQ * (NPP + HALO)], BF16, name=f"stg{c}")
                sv = stg.rearrange("p (q n) -> p q n", q=NQ)
                src = bass.AP(
                    tensor=scr.tensor,
                    offset=scr.offset + l0,
                    ap=[[NPP, 64], [SCR_W, NQ], [1, NPP + HALO]],
                )
                nc.sync.dma_start(sv, src)

                win = fin.tile([64, NQ * NPP], BF16, name=f"win{c}")
                wv = win.rearrange("p (q n) -> p q n", q=NQ)
                nc.vector.tensor_add(wv, sv[:, :, 0:NPP], sv[:, :, 1:NPP + 1])
                for j in range(2, K):
                    nc.vector.tensor_add(wv, wv, sv[:, :, j:NPP + j])

                def winq(q):
                    return win[:, q * NPP:(q + 1) * NPP]
                n2 = fin.tile([64, NPP], BF16, name=f"n2_{c}")
                t2 = fin.tile([64, NPP], BF16, name=f"t2_{c}")
                d2 = fin.tile([64, NPP], BF16, name=f"d2_{c}")
                rd = fin.tile([64, NPP], F32, name=f"rd_{c}")
                nc.vector.tensor_mul(n2[:, :], winq(0), winq(0))
                nc.vector.tensor_mul(t2[:, :], winq(1), winq(1))
                nc.vector.tensor_add(n2[:, :], n2[:, :], t2[:, :])
                nc.vector.tensor_mul(d2[:, :], winq(2), winq(3))
                nc.vector.reciprocal(rd[:, :], d2[:, :])
                nc.vector.tensor_mul(n2[:, :], n2[:, :], rd[:, :])
                if c == 3:
                    nc.vector.tensor_mul(n2[:, :], n2[:, :], mask[:, :])
                sq = fin.tile([64, NPP], F32, name=f"sq{c}")
                acc = fin.tile([64, 1], F32, name=f"acc{c}")
                nc.scalar.activation(
                    sq[:, :], n2[:, :],
                    mybir.ActivationFunctionType.Sqrt,
                    accum_out=acc[:, :],
                )
                nc.scalar.dma_start(out_d[0:64, c:c + 1], acc[:, :])

            # chunk c covers l in [l0, l0+4096): c0: [0,4096) needs groups
            # 0-3 h0 (+4-col halo from g4 h0); c1: [8192, 12288) g0-3 h1
            # (+halo g4 h1); c2, c3 the rest.
            for g in range(NG):
                emit_group(g)
                if g == 4:
                    emit_chunk(0)
                    emit_chunk(1)
            emit_chunk(2)
            emit_chunk(3)

    _fuse_ldweights(nc)
    nc.compile()
    return nc


_NC = None


def _get_nc() -> bacc.Bacc:
    global _NC
    if _NC is None:
        _NC = build_nc()
    return _NC


def kernel(pred_real, pred_imag, targ_real, targ_imag, filter_size=5):
    global LAST_RESULT
    assert int(filter_size) == K
    nc = _get_nc()

    in_maps = []
    for b in range(B):
        # plane-separate: partition p = c + 64h, plane cols = l % 8192;
        # group-blocked [PR | PI | TR | TI] per 1024-col stripe.
        arr = np.empty((P, 4 * HL), dtype=NP_BF16)
        a4 = arr.reshape(P, NG, 4, FD)
        for j, x in enumerate((pred_real[b], pred_imag[b],
                               targ_real[b], targ_imag[b])):
            xp = np.asarray(x, dtype=np.float32).reshape(C, 2, NG, FD)
            # -> [h*64 + c, g, n]
            a4[:, :, j, :] = xp.transpose(1, 0, 2, 3).reshape(P, NG, FD) \
                .astype(NP_BF16)
        in_maps.append({"inp": arr})

    kwargs = {}
    if PROFILE:
        kwargs = dict(trace=True)
        if TRACE_DIR is not None:
            import os
            os.makedirs(TRACE_DIR, exist_ok=True)
            kwargs["tmpdir"] = TRACE_DIR
    res = run_bass_kernel_spmd(nc, in_maps, core_ids=list(range(B)), **kwargs)
    LAST_RESULT = res

    total = 0.0
    for r in res.results:
        total += np.asarray(r["out"], dtype=np.float64)[0:64, :].sum()
    coh = total / (B * NVALID)
    return np.float32(1.0 - coh)
